# revision 8
# baseline (speedup 1.0000x reference)
"""Paged GQA decode attention (B=64, HQ=32, HKV=8, D=128) on 8 TRN2 NeuronCores.

Strategy: data-parallel over requests with host-side load balancing.
 - Sort the 64 requests by context_lens descending; slot r of core c gets the
   rank-(r*8+c) request, so every core's slot-r request has a similar length.
 - Each slot is padded to the max-of-8 chunk count (chunks of 128 tokens), so
   all 8 cores execute the SAME static program (SPMD) on different data.
 - Host gathers each request's KV blocks (honoring block_tables) into per-core
   shards: K pre-transposed to [d, l] tiles (no on-chip transposes), V natural
   [l, d]; both bf16. The token-validity mask is folded into V host-side
   (invalid rows zeroed) and each kv head's V carries a 129th column holding
   the validity indicator, so the PV matmul also accumulates the softmax
   denominator and the exp bias is a compile-time constant.
 - K and V for GRP chunks are packed into ONE combined [128, ...] DMA group
   (~4MB) for near-peak SDMA efficiency; group DMAs round-robin across the
   gpsimd/scalar/sync issue queues.
 - Per chunk on device: scores[l,hq] = K_h^T.T @ qT (8 matmuls) into a shared
   PSUM batch tile; one exp activation per EBATCH chunks on ScalarE; then PV
   accumulation acc[hq,d+1] += E_h.T @ V_h (8 col-tiled matmuls into two PSUM
   banks). PSUM drains go through VectorE. Final division happens on host.
"""

import math
import os
import sys
from contextlib import ExitStack

import numpy as np
import ml_dtypes  # noqa: F401  (numpy bf16/fp8 dtypes)

for _p in ("/opt/trn_rl_repo", "/root/.axon_site/_ro/trn_rl_repo"):
    if os.path.isdir(_p) and _p not in sys.path:
        sys.path.insert(0, _p)
        break

import concourse.bass as bass  # noqa: F401
import concourse.tile as tile
from concourse import bacc, mybir
from concourse.bass_utils import run_bass_kernel_spmd

B, HQ, HKV, D, BS, MB = 64, 32, 8, 128, 16, 128
G = HQ // HKV              # 4 query heads per kv head
SCALE = 0.08838834764831845
NCORES = 8
SLOTS = B // NCORES        # 8 request slots per core
CHUNK = 128                # tokens per chunk (= SBUF partitions)
BPC = CHUNK // BS          # blocks per chunk = 8
ROW = HKV * D              # 1024 K elements per token row
DV = D + 1                 # V head row + denominator indicator column
ROWV = HKV * DV            # 1032 V elements per token row
VSHIFT = -2.0              # shift scores so exp() stays well-ranged
GRP = 8                    # chunks per combined K+V DMA group (~4MB)
KV_BUFS = 5                # group tiles in flight
EBATCH = 8                 # chunks per exp activation (divides GRP)
K_DT = "bf16"              # K/q dtype
V_DT = "bf16"              # V/E dtype

last_results = None        # stashed BassKernelResults for test.py

_prog_cache = {}


def _mdt(name):
    return {"f32": mybir.dt.float32, "bf16": mybir.dt.bfloat16,
            "fp8": mybir.dt.float8e4}[name]


def _ndt(name):
    return mybir.dt.np(_mdt(name))


def _group_sizes(C_total):
    """Split C_total chunks into GRP-sized DMA groups (smaller tail ok)."""
    sizes = [GRP] * (C_total // GRP)
    if C_total % GRP:
        sizes.append(C_total % GRP)
    return sizes


def _build_program(s_counts):
    f32 = mybir.dt.float32
    kdt, vdt = _mdt(K_DT), _mdt(V_DT)
    C_total = sum(s_counts)
    gsizes = _group_sizes(C_total)
    goff = np.cumsum([0] + [s * (ROW + ROWV) for s in gsizes])
    slot_of = []
    for r, s in enumerate(s_counts):
        slot_of += [r] * s
    nc = bacc.Bacc()
    vsh = nc.alloc_sbuf_tensor("const-f32-vshift", [128, 1], f32)
    nc.gpsimd.memset(vsh.ap(), float(VSHIFT))
    nc.const_aps.aps[(f32, float(VSHIFT))] = vsh.ap()
    nc.all_engine_barrier()

    kv_d = nc.declare_dram_parameter("kv", [D, int(goff[-1])], kdt,
                                     isOutput=False)
    qT_d = nc.declare_dram_parameter("qT", [D, SLOTS * HQ], kdt, isOutput=False)
    out_d = nc.declare_dram_parameter("out", [SLOTS, 2, CHUNK, DV], f32,
                                      isOutput=True)

    EXP = mybir.ActivationFunctionType.Exp

    with tile.TileContext(nc) as tc, ExitStack() as ctx:
        kvpool = ctx.enter_context(tc.tile_pool(name="kvp", bufs=KV_BUFS))
        epool = ctx.enter_context(tc.tile_pool(name="e", bufs=3))
        const = ctx.enter_context(tc.tile_pool(name="cst", bufs=1))
        spsum = ctx.enter_context(tc.tile_pool(name="sp", bufs=2, space="PSUM"))
        apsum = ctx.enter_context(tc.tile_pool(name="ac", bufs=2, space="PSUM"))

        q_all = const.tile([D, SLOTS * HQ], kdt)
        nc.sync.dma_start(q_all[:], qT_d[:])
        # dummy matmul absorbs the q_all DMA wait so the first real matmul
        # only waits on its k/v DMA.
        dmy = spsum.tile([1, 1], f32, tag="sco")
        nc.tensor.matmul(dmy[:], q_all[0:1, 0:1], q_all[0:1, 0:1],
                         start=True, stop=True)

        bulk_engs = (nc.gpsimd, nc.scalar, nc.sync)
        cur = {}
        accs = {}

        def get_acc(r):
            if r not in accs:
                accs[r] = (apsum.tile([CHUNK, DV], f32, tag="acca", name="acca"),
                           apsum.tile([CHUNK, DV], f32, tag="accb", name="accb"))
            return accs[r]

        def drain_slot(r):
            acc_a, acc_b = accs.pop(r)
            out_sa = epool.tile([CHUNK, DV], f32, tag="outa")
            out_sb = epool.tile([CHUNK, DV], f32, tag="outb")
            nc.vector.tensor_copy(out_sa[:], acc_a[:])
            nc.vector.tensor_copy(out_sb[:], acc_b[:])
            nc.sync.dma_start(out_d[r, 0], out_sa[:])
            nc.sync.dma_start(out_d[r, 1], out_sb[:])

        def emit_pv(pend):
            b0, bs, et, kv_t, g_sz = pend
            voff = g_sz * ROW
            for bi in range(bs):
                idx = b0 + bi
                half = idx - (idx // GRP) * GRP
                vt = kv_t[:, voff + half * ROWV:voff + (half + 1) * ROWV]
                r = slot_of[idx]
                st = idx == 0 or slot_of[idx - 1] != r
                sp = idx == C_total - 1 or slot_of[idx + 1] != r
                acc_a, acc_b = get_acc(r)
                for h in range(HKV):
                    accp = acc_a if h < 4 else acc_b
                    jj = h % 4
                    nc.tensor.matmul(
                        accp[32 * jj:32 * jj + G, :],
                        et[:, bi * HQ + h * G:bi * HQ + (h + 1) * G],
                        vt[:, h * DV:(h + 1) * DV],
                        start=st, stop=sp,
                        tile_position=(0, 32 * jj),
                    )
                if sp:
                    drain_slot(r)

        # software-pipelined: QK+exp of batch b+1 are emitted before PV of
        # batch b, so the PE never stalls waiting for ScalarE's exp.
        pending = None
        for b0 in range(0, C_total, EBATCH):
            bs = min(EBATCH, C_total - b0)
            sco = spsum.tile([CHUNK, bs * HQ], f32, tag="sco")
            for bi in range(bs):
                idx = b0 + bi
                g, half = divmod(idx, GRP)
                if half == 0 or "kv" not in cur:
                    g_sz = gsizes[g]
                    cur["kv"] = kvpool.tile([D, g_sz * (ROW + ROWV)], kdt,
                                            tag="kvg", name="kvg")
                    cur["gsz"] = g_sz
                    # split each group across the 3 DMA issue queues so the
                    # SDMA engines' packet round-robin works on ONE group at
                    # a time and groups complete in order
                    ncols = g_sz * (ROW + ROWV)
                    c0, c1, c2 = 0, ncols // 3, 2 * ncols // 3
                    for pi, (lo, hi) in enumerate(
                            ((c0, c1), (c1, c2), (c2, ncols))):
                        bulk_engs[pi].dma_start(
                            cur["kv"][:, lo:hi],
                            kv_d[:, int(goff[g]) + lo:int(goff[g]) + hi])
                kt = cur["kv"][:, half * ROW:(half + 1) * ROW]
                r = slot_of[idx]
                qt = q_all[:, r * HQ:(r + 1) * HQ]
                for h in range(HKV):
                    nc.tensor.matmul(
                        sco[:, bi * HQ + h * G:bi * HQ + (h + 1) * G],
                        kt[:, h * D:(h + 1) * D],
                        qt[:, h * G:(h + 1) * G],
                        start=True, stop=True,
                    )
            et = epool.tile([CHUNK, bs * HQ], vdt)
            nc.scalar.activation(et[:], sco[:], EXP, bias=VSHIFT, scale=1.0)
            if pending is not None:
                emit_pv(pending)
            pending = (b0, bs, et, cur["kv"], cur["gsz"])
        emit_pv(pending)
    nc.compile()
    return nc


def _get_program(s_counts):
    if s_counts not in _prog_cache:
        _prog_cache[s_counts] = _build_program(s_counts)
    return _prog_cache[s_counts]


def _make_schedule(context_lens):
    L = context_lens.astype(np.int64)
    order = np.argsort(-L, kind="stable")
    s_counts = []
    for r in range(SLOTS):
        grp = order[r * NCORES:(r + 1) * NCORES]
        s_counts.append(max(1, math.ceil(int(L[grp].max()) / CHUNK)))
    return order, tuple(s_counts)


def _build_in_maps(q, k_cache, v_cache, block_tables, L, order, s_counts):
    np_k, np_v = _ndt(K_DT), _ndt(V_DT)
    C_total = sum(s_counts)
    gsizes = _group_sizes(C_total)
    nblocks_total = k_cache.shape[0]
    kf = k_cache.reshape(nblocks_total, BS, ROW)
    vf = v_cache.reshape(nblocks_total, BS, HKV, D)

    in_maps = []
    core_reqs = []
    for c in range(NCORES):
        karr = np.empty((C_total, D, ROW), np_k)
        varr = np.zeros((C_total, CHUNK, HKV, DV), np_v)
        qT = np.empty((D, SLOTS * HQ), np_k)
        reqs = []
        gc = 0
        for r in range(SLOTS):
            b = int(order[r * NCORES + c])
            reqs.append(b)
            S_r = s_counts[r]
            blocks = np.clip(block_tables[b, :S_r * BPC].astype(np.int64),
                             0, nblocks_total - 1)
            kreq = kf[blocks].reshape(S_r, CHUNK, HKV, D)
            karr[gc:gc + S_r] = \
                kreq.transpose(0, 3, 2, 1).reshape(S_r, D, ROW)
            Lb = int(L[b])
            nval = min(S_r * CHUNK, Lb)  # valid tokens in this slot
            vreq = vf[blocks].reshape(S_r * CHUNK, HKV, D)
            va = varr[gc:gc + S_r].reshape(S_r * CHUNK, HKV, DV)
            va[:nval, :, :D] = vreq[:nval]
            va[:nval, :, D] = 1.0
            qT[:, r * HQ:(r + 1) * HQ] = (q[b] * SCALE).T
            gc += S_r
        # pack combined K+V DMA groups: [D, (K chunks...)(V chunks...)] per
        # group, groups concatenated along the free dim
        varr2 = varr.reshape(C_total, CHUNK, ROWV)
        parts = []
        gc2 = 0
        for g_sz in gsizes:
            kg = karr[gc2:gc2 + g_sz].transpose(1, 0, 2).reshape(D, g_sz * ROW)
            vg = varr2[gc2:gc2 + g_sz].transpose(1, 0, 2) \
                .reshape(D, g_sz * ROWV)
            parts.append(kg)
            parts.append(vg)
            gc2 += g_sz
        kvh = np.ascontiguousarray(np.concatenate(parts, axis=1))
        in_maps.append({"kv": kvh, "qT": qT})
        core_reqs.append(reqs)
    return in_maps, core_reqs


def kernel(q, k_cache, v_cache, block_tables, context_lens):
    global last_results
    q = np.asarray(q, dtype=np.float32)
    k_cache = np.asarray(k_cache, dtype=np.float32)
    v_cache = np.asarray(v_cache, dtype=np.float32)
    block_tables = np.asarray(block_tables, dtype=np.int32)
    context_lens = np.asarray(context_lens, dtype=np.int32)

    L = context_lens.astype(np.int64)
    order, s_counts = _make_schedule(context_lens)
    nc = _get_program(s_counts)
    in_maps, core_reqs = _build_in_maps(
        q, k_cache, v_cache, block_tables, L, order, s_counts)

    res = run_bass_kernel_spmd(
        nc, in_maps, list(range(NCORES)),
        trace=bool(os.environ.get("KBASS_TRACE")),
    )
    last_results = res

    out = np.empty((B, HQ, D), np.float32)
    for c in range(NCORES):
        full = res.results[c]["out"].reshape(SLOTS, 2, CHUNK, DV)
        for r, b in enumerate(core_reqs[c]):
            acc = np.empty((HQ, DV), np.float32)
            for h in range(HKV):
                jj = h % 4
                acc[h * G:(h + 1) * G] = \
                    full[r, 0 if h < 4 else 1, 32 * jj:32 * jj + G, :]
            den = np.maximum(acc[:, D:], 1e-30)
            out[b] = acc[:, :D] / den
    return out


# revision 9
# speedup vs baseline: 1.1107x; 1.1107x over previous
"""Paged GQA decode attention (B=64, HQ=32, HKV=8, D=128) on 8 TRN2 NeuronCores.

Strategy: data-parallel over requests with host-side load balancing.
 - Sort the 64 requests by context_lens descending; slot r of core c gets the
   rank-(r*8+c) request, so every core's slot-r request has a similar length.
 - Each slot is padded to the max-of-8 chunk count (chunks of 128 tokens), so
   all 8 cores execute the SAME static program (SPMD) on different data.
 - Host gathers each request's KV blocks (honoring block_tables) into per-core
   shards: K pre-transposed to [d, l] tiles (no on-chip transposes), V natural
   [l, d]; both bf16. The token-validity mask is folded into V host-side
   (invalid rows zeroed) and each kv head's V carries a 129th column holding
   the validity indicator, so the PV matmul also accumulates the softmax
   denominator and the exp bias is a compile-time constant.
 - K and V for GRP chunks are packed into ONE combined [128, ...] DMA group
   (~4MB) for near-peak SDMA efficiency; group DMAs round-robin across the
   gpsimd/scalar/sync issue queues.
 - Per chunk on device: scores[l,hq] = K_h^T.T @ qT (8 matmuls) into a shared
   PSUM batch tile; one exp activation per EBATCH chunks on ScalarE; then PV
   accumulation acc[hq,d+1] += E_h.T @ V_h (8 col-tiled matmuls into two PSUM
   banks). PSUM drains go through VectorE. Final division happens on host.
"""

import math
import os
import sys
from contextlib import ExitStack

import numpy as np
import ml_dtypes  # noqa: F401  (numpy bf16/fp8 dtypes)

for _p in ("/opt/trn_rl_repo", "/root/.axon_site/_ro/trn_rl_repo"):
    if os.path.isdir(_p) and _p not in sys.path:
        sys.path.insert(0, _p)
        break

import concourse.bass as bass  # noqa: F401
import concourse.tile as tile
from concourse import bacc, mybir
from concourse.bass_utils import run_bass_kernel_spmd

B, HQ, HKV, D, BS, MB = 64, 32, 8, 128, 16, 128
G = HQ // HKV              # 4 query heads per kv head
SCALE = 0.08838834764831845
NCORES = 8
SLOTS = B // NCORES        # 8 request slots per core
CHUNK = 128                # tokens per chunk (= SBUF partitions)
BPC = CHUNK // BS          # blocks per chunk = 8
ROW = HKV * D              # 1024 K elements per token row
DV = D + 1                 # V head row + denominator indicator column
ROWV = HKV * DV            # 1032 V elements per token row
VSHIFT = -2.0              # shift scores so exp() stays well-ranged
GRP = 8                    # chunks per combined K+V DMA group (~4MB)
KV_BUFS = 5                # group tiles in flight
EBATCH = 8                 # chunks per exp activation (divides GRP)
K_DT = "bf16"              # K/q dtype
V_DT = "bf16"              # V/E dtype

last_results = None        # stashed BassKernelResults for test.py

_prog_cache = {}


def _mdt(name):
    return {"f32": mybir.dt.float32, "bf16": mybir.dt.bfloat16,
            "fp8": mybir.dt.float8e4}[name]


def _ndt(name):
    return mybir.dt.np(_mdt(name))


def _group_sizes(C_total):
    """Split C_total chunks into GRP-sized DMA groups (smaller tail ok)."""
    sizes = [GRP] * (C_total // GRP)
    if C_total % GRP:
        sizes.append(C_total % GRP)
    return sizes


def _build_program(s_counts):
    f32 = mybir.dt.float32
    kdt, vdt = _mdt(K_DT), _mdt(V_DT)
    C_total = sum(s_counts)
    gsizes = _group_sizes(C_total)
    goff = np.cumsum([0] + [s * (ROW + ROWV) for s in gsizes])
    slot_of = []
    for r, s in enumerate(s_counts):
        slot_of += [r] * s
    nc = bacc.Bacc()
    vsh = nc.alloc_sbuf_tensor("const-f32-vshift", [128, 1], f32)
    nc.gpsimd.memset(vsh.ap(), float(VSHIFT))
    nc.const_aps.aps[(f32, float(VSHIFT))] = vsh.ap()
    nc.all_engine_barrier()

    kv_d = nc.declare_dram_parameter("kv", [D, int(goff[-1])], kdt,
                                     isOutput=False)
    qT_d = nc.declare_dram_parameter("qT", [D, SLOTS * HQ], kdt, isOutput=False)
    out_d = nc.declare_dram_parameter("out", [SLOTS, 2, CHUNK, DV], f32,
                                      isOutput=True)

    EXP = mybir.ActivationFunctionType.Exp

    with tile.TileContext(nc) as tc, ExitStack() as ctx:
        kvpool = ctx.enter_context(tc.tile_pool(name="kvp", bufs=KV_BUFS))
        epool = ctx.enter_context(tc.tile_pool(name="e", bufs=3))
        const = ctx.enter_context(tc.tile_pool(name="cst", bufs=1))
        spsum = ctx.enter_context(tc.tile_pool(name="sp", bufs=2, space="PSUM"))
        apsum = ctx.enter_context(tc.tile_pool(name="ac", bufs=2, space="PSUM"))

        q_all = const.tile([D, SLOTS * HQ], kdt)
        nc.sync.dma_start(q_all[:], qT_d[:])
        # dummy matmul absorbs the q_all DMA wait so the first real matmul
        # only waits on its k/v DMA.
        dmy = spsum.tile([1, 1], f32, tag="sco")
        nc.tensor.matmul(dmy[:], q_all[0:1, 0:1], q_all[0:1, 0:1],
                         start=True, stop=True)

        bulk_engs = (nc.gpsimd, nc.scalar, nc.sync)
        cur = {}
        accs = {}

        def get_acc(r):
            if r not in accs:
                accs[r] = (apsum.tile([CHUNK, DV], f32, tag="acca", name="acca"),
                           apsum.tile([CHUNK, DV], f32, tag="accb", name="accb"))
            return accs[r]

        def drain_slot(r):
            acc_a, acc_b = accs.pop(r)
            out_sa = epool.tile([CHUNK, DV], f32, tag="outa")
            out_sb = epool.tile([CHUNK, DV], f32, tag="outb")
            nc.vector.tensor_copy(out_sa[:], acc_a[:])
            nc.vector.tensor_copy(out_sb[:], acc_b[:])
            nc.sync.dma_start(out_d[r, 0], out_sa[:])
            nc.sync.dma_start(out_d[r, 1], out_sb[:])

        def emit_pv(pend):
            b0, bs, et, kv_t, g_sz = pend
            voff = g_sz * ROW
            for bi in range(bs):
                idx = b0 + bi
                half = idx - (idx // GRP) * GRP
                vt = kv_t[:, voff + half * ROWV:voff + (half + 1) * ROWV]
                r = slot_of[idx]
                st = idx == 0 or slot_of[idx - 1] != r
                sp = idx == C_total - 1 or slot_of[idx + 1] != r
                acc_a, acc_b = get_acc(r)
                for h in range(HKV):
                    accp = acc_a if h < 4 else acc_b
                    jj = h % 4
                    nc.tensor.matmul(
                        accp[32 * jj:32 * jj + G, :],
                        et[:, bi * HQ + h * G:bi * HQ + (h + 1) * G],
                        vt[:, h * DV:(h + 1) * DV],
                        start=st, stop=sp,
                        tile_position=(0, 32 * jj),
                    )
                if sp:
                    drain_slot(r)

        # software-pipelined: QK+exp of batch b+1 are emitted before PV of
        # batch b, so the PE never stalls waiting for ScalarE's exp.
        pending = None
        for b0 in range(0, C_total, EBATCH):
            bs = min(EBATCH, C_total - b0)
            sco = spsum.tile([CHUNK, bs * HQ], f32, tag="sco")
            for bi in range(bs):
                idx = b0 + bi
                g, half = divmod(idx, GRP)
                if half == 0 or "kv" not in cur:
                    g_sz = gsizes[g]
                    cur["kv"] = kvpool.tile([D, g_sz * (ROW + ROWV)], kdt,
                                            tag="kvg", name="kvg")
                    cur["gsz"] = g_sz
                    # all bulk groups on ONE queue: per-queue FIFO makes
                    # groups complete in order (the engines' packet
                    # round-robin across queues would finish concurrent
                    # groups simultaneously, stalling the first compute)
                    nc.gpsimd.dma_start(
                        cur["kv"][:], kv_d[:, int(goff[g]):int(goff[g + 1])])
                kt = cur["kv"][:, half * ROW:(half + 1) * ROW]
                r = slot_of[idx]
                qt = q_all[:, r * HQ:(r + 1) * HQ]
                for h in range(HKV):
                    nc.tensor.matmul(
                        sco[:, bi * HQ + h * G:bi * HQ + (h + 1) * G],
                        kt[:, h * D:(h + 1) * D],
                        qt[:, h * G:(h + 1) * G],
                        start=True, stop=True,
                    )
            et = epool.tile([CHUNK, bs * HQ], vdt)
            nc.scalar.activation(et[:], sco[:], EXP, bias=VSHIFT, scale=1.0)
            if pending is not None:
                emit_pv(pending)
            pending = (b0, bs, et, cur["kv"], cur["gsz"])
        emit_pv(pending)
    nc.compile()
    return nc


def _get_program(s_counts):
    if s_counts not in _prog_cache:
        _prog_cache[s_counts] = _build_program(s_counts)
    return _prog_cache[s_counts]


def _make_schedule(context_lens):
    L = context_lens.astype(np.int64)
    order = np.argsort(-L, kind="stable")
    s_counts = []
    for r in range(SLOTS):
        grp = order[r * NCORES:(r + 1) * NCORES]
        s_counts.append(max(1, math.ceil(int(L[grp].max()) / CHUNK)))
    return order, tuple(s_counts)


def _build_in_maps(q, k_cache, v_cache, block_tables, L, order, s_counts):
    np_k, np_v = _ndt(K_DT), _ndt(V_DT)
    C_total = sum(s_counts)
    gsizes = _group_sizes(C_total)
    nblocks_total = k_cache.shape[0]
    kf = k_cache.reshape(nblocks_total, BS, ROW)
    vf = v_cache.reshape(nblocks_total, BS, HKV, D)

    in_maps = []
    core_reqs = []
    for c in range(NCORES):
        karr = np.empty((C_total, D, ROW), np_k)
        varr = np.zeros((C_total, CHUNK, HKV, DV), np_v)
        qT = np.empty((D, SLOTS * HQ), np_k)
        reqs = []
        gc = 0
        for r in range(SLOTS):
            b = int(order[r * NCORES + c])
            reqs.append(b)
            S_r = s_counts[r]
            blocks = np.clip(block_tables[b, :S_r * BPC].astype(np.int64),
                             0, nblocks_total - 1)
            kreq = kf[blocks].reshape(S_r, CHUNK, HKV, D)
            karr[gc:gc + S_r] = \
                kreq.transpose(0, 3, 2, 1).reshape(S_r, D, ROW)
            Lb = int(L[b])
            nval = min(S_r * CHUNK, Lb)  # valid tokens in this slot
            vreq = vf[blocks].reshape(S_r * CHUNK, HKV, D)
            va = varr[gc:gc + S_r].reshape(S_r * CHUNK, HKV, DV)
            va[:nval, :, :D] = vreq[:nval]
            va[:nval, :, D] = 1.0
            qT[:, r * HQ:(r + 1) * HQ] = (q[b] * SCALE).T
            gc += S_r
        # pack combined K+V DMA groups: [D, (K chunks...)(V chunks...)] per
        # group, groups concatenated along the free dim
        varr2 = varr.reshape(C_total, CHUNK, ROWV)
        parts = []
        gc2 = 0
        for g_sz in gsizes:
            kg = karr[gc2:gc2 + g_sz].transpose(1, 0, 2).reshape(D, g_sz * ROW)
            vg = varr2[gc2:gc2 + g_sz].transpose(1, 0, 2) \
                .reshape(D, g_sz * ROWV)
            parts.append(kg)
            parts.append(vg)
            gc2 += g_sz
        kvh = np.ascontiguousarray(np.concatenate(parts, axis=1))
        in_maps.append({"kv": kvh, "qT": qT})
        core_reqs.append(reqs)
    return in_maps, core_reqs


def kernel(q, k_cache, v_cache, block_tables, context_lens):
    global last_results
    q = np.asarray(q, dtype=np.float32)
    k_cache = np.asarray(k_cache, dtype=np.float32)
    v_cache = np.asarray(v_cache, dtype=np.float32)
    block_tables = np.asarray(block_tables, dtype=np.int32)
    context_lens = np.asarray(context_lens, dtype=np.int32)

    L = context_lens.astype(np.int64)
    order, s_counts = _make_schedule(context_lens)
    nc = _get_program(s_counts)
    in_maps, core_reqs = _build_in_maps(
        q, k_cache, v_cache, block_tables, L, order, s_counts)

    res = run_bass_kernel_spmd(
        nc, in_maps, list(range(NCORES)),
        trace=bool(os.environ.get("KBASS_TRACE")),
    )
    last_results = res

    out = np.empty((B, HQ, D), np.float32)
    for c in range(NCORES):
        full = res.results[c]["out"].reshape(SLOTS, 2, CHUNK, DV)
        for r, b in enumerate(core_reqs[c]):
            acc = np.empty((HQ, DV), np.float32)
            for h in range(HKV):
                jj = h % 4
                acc[h * G:(h + 1) * G] = \
                    full[r, 0 if h < 4 else 1, 32 * jj:32 * jj + G, :]
            den = np.maximum(acc[:, D:], 1e-30)
            out[b] = acc[:, :D] / den
    return out


# revision 11
# speedup vs baseline: 1.1798x; 1.0622x over previous
"""Paged GQA decode attention (B=64, HQ=32, HKV=8, D=128) on 8 TRN2 NeuronCores.

Strategy: data-parallel over requests with host-side load balancing.
 - Sort the 64 requests by context_lens descending; slot r of core c gets the
   rank-(r*8+c) request, so every core's slot-r request has a similar length.
 - Each slot is padded to the max-of-8 chunk count (chunks of 128 tokens), so
   all 8 cores execute the SAME static program (SPMD) on different data.
 - Host gathers each request's KV blocks (honoring block_tables) into per-core
   shards: K pre-transposed to [d, l] tiles (no on-chip transposes), V natural
   [l, d]; both bf16. The token-validity mask is folded into V host-side
   (invalid rows zeroed) and each kv head's V carries a 129th column holding
   the validity indicator, so the PV matmul also accumulates the softmax
   denominator and the exp bias is a compile-time constant.
 - K and V for GRP chunks are packed into ONE combined [128, ...] DMA group
   (~4MB) for near-peak SDMA efficiency; group DMAs round-robin across the
   gpsimd/scalar/sync issue queues.
 - Per chunk on device: scores[l,hq] = K_h^T.T @ qT (8 matmuls) into a shared
   PSUM batch tile; one exp activation per EBATCH chunks on ScalarE; then PV
   accumulation acc[hq,d+1] += E_h.T @ V_h (8 col-tiled matmuls into two PSUM
   banks). PSUM drains go through VectorE. Final division happens on host.
"""

import math
import os
import sys
from contextlib import ExitStack

import numpy as np
import ml_dtypes  # noqa: F401  (numpy bf16/fp8 dtypes)

for _p in ("/opt/trn_rl_repo", "/root/.axon_site/_ro/trn_rl_repo"):
    if os.path.isdir(_p) and _p not in sys.path:
        sys.path.insert(0, _p)
        break

import concourse.bass as bass  # noqa: F401
import concourse.tile as tile
from concourse import bacc, mybir
from concourse.bass_utils import run_bass_kernel_spmd

B, HQ, HKV, D, BS, MB = 64, 32, 8, 128, 16, 128
G = HQ // HKV              # 4 query heads per kv head
SCALE = 0.08838834764831845
NCORES = 8
SLOTS = B // NCORES        # 8 request slots per core
CHUNK = 128                # tokens per chunk (= SBUF partitions)
BPC = CHUNK // BS          # blocks per chunk = 8
ROW = HKV * D              # 1024 K elements per token row
DV = D + 1                 # V head row + denominator indicator column
ROWV = HKV * DV            # 1032 V elements per token row
VSHIFT = -2.0              # shift scores so exp() stays well-ranged
GRP = 8                    # chunks per combined K+V DMA group (~4MB)
KV_BUFS = 5                # group tiles in flight
EBATCH = 8                 # chunks per exp activation (divides GRP)
K_DT = "bf16"              # K/q dtype
V_DT = "bf16"              # V/E dtype

last_results = None        # stashed BassKernelResults for test.py

_prog_cache = {}


def _mdt(name):
    return {"f32": mybir.dt.float32, "bf16": mybir.dt.bfloat16,
            "fp8": mybir.dt.float8e4}[name]


def _ndt(name):
    return mybir.dt.np(_mdt(name))


def _group_sizes(C_total):
    """Split C_total chunks into GRP-sized DMA groups (smaller tail ok)."""
    sizes = [GRP] * (C_total // GRP)
    if C_total % GRP:
        sizes.append(C_total % GRP)
    return sizes


def _build_program(s_counts):
    f32 = mybir.dt.float32
    kdt, vdt = _mdt(K_DT), _mdt(V_DT)
    C_total = sum(s_counts)
    gsizes = _group_sizes(C_total)
    goff = np.cumsum([0] + [s * (ROW + ROWV) for s in gsizes])
    slot_of = []
    for r, s in enumerate(s_counts):
        slot_of += [r] * s
    nc = bacc.Bacc()
    vsh = nc.alloc_sbuf_tensor("const-f32-vshift", [128, 1], f32)
    nc.gpsimd.memset(vsh.ap(), float(VSHIFT))
    nc.const_aps.aps[(f32, float(VSHIFT))] = vsh.ap()
    nc.all_engine_barrier()

    kv_d = nc.declare_dram_parameter("kv", [D, int(goff[-1])], kdt,
                                     isOutput=False)
    qT_d = nc.declare_dram_parameter("qT", [D, SLOTS * HQ], kdt, isOutput=False)
    out_d = nc.declare_dram_parameter("out", [CHUNK, SLOTS * 2 * DV], f32,
                                      isOutput=True)

    EXP = mybir.ActivationFunctionType.Exp

    with tile.TileContext(nc) as tc, ExitStack() as ctx:
        kvpool = ctx.enter_context(tc.tile_pool(name="kvp", bufs=KV_BUFS))
        epool = ctx.enter_context(tc.tile_pool(name="e", bufs=3))
        const = ctx.enter_context(tc.tile_pool(name="cst", bufs=1))
        spsum = ctx.enter_context(tc.tile_pool(name="sp", bufs=2, space="PSUM"))
        apsum = ctx.enter_context(tc.tile_pool(name="ac", bufs=2, space="PSUM"))

        q_all = const.tile([D, SLOTS * HQ], kdt)
        nc.sync.dma_start(q_all[:], qT_d[:])
        out_stage = const.tile([CHUNK, SLOTS * 2 * DV], f32)
        # dummy matmul absorbs the q_all DMA wait so the first real matmul
        # only waits on its k/v DMA.
        dmy = spsum.tile([1, 1], f32, tag="sco")
        nc.tensor.matmul(dmy[:], q_all[0:1, 0:1], q_all[0:1, 0:1],
                         start=True, stop=True)

        bulk_engs = (nc.gpsimd, nc.scalar, nc.sync)
        cur = {}
        accs = {}

        def get_acc(r):
            if r not in accs:
                accs[r] = (apsum.tile([CHUNK, DV], f32, tag="acca", name="acca"),
                           apsum.tile([CHUNK, DV], f32, tag="accb", name="accb"))
            return accs[r]

        def drain_slot(r):
            acc_a, acc_b = accs.pop(r)
            o = r * 2 * DV
            nc.vector.tensor_copy(out_stage[:, o:o + DV], acc_a[:])
            nc.vector.tensor_copy(out_stage[:, o + DV:o + 2 * DV], acc_b[:])

        def emit_pv(pend):
            b0, bs, et, kv_t, g_sz = pend
            voff = g_sz * ROW
            for bi in range(bs):
                idx = b0 + bi
                half = idx - (idx // GRP) * GRP
                vt = kv_t[:, voff + half * ROWV:voff + (half + 1) * ROWV]
                r = slot_of[idx]
                st = idx == 0 or slot_of[idx - 1] != r
                sp = idx == C_total - 1 or slot_of[idx + 1] != r
                acc_a, acc_b = get_acc(r)
                for h in range(HKV):
                    accp = acc_a if h < 4 else acc_b
                    jj = h % 4
                    nc.tensor.matmul(
                        accp[32 * jj:32 * jj + G, :],
                        et[:, bi * HQ + h * G:bi * HQ + (h + 1) * G],
                        vt[:, h * DV:(h + 1) * DV],
                        start=st, stop=sp,
                        tile_position=(0, 32 * jj),
                    )
                if sp:
                    drain_slot(r)

        # software-pipelined: QK+exp of batch b+1 are emitted before PV of
        # batch b, so the PE never stalls waiting for ScalarE's exp.
        pending = None
        for b0 in range(0, C_total, EBATCH):
            bs = min(EBATCH, C_total - b0)
            sco = spsum.tile([CHUNK, bs * HQ], f32, tag="sco")
            for bi in range(bs):
                idx = b0 + bi
                g, half = divmod(idx, GRP)
                if half == 0 or "kv" not in cur:
                    g_sz = gsizes[g]
                    cur["kv"] = kvpool.tile([D, g_sz * (ROW + ROWV)], kdt,
                                            tag="kvg", name="kvg")
                    cur["gsz"] = g_sz
                    # all bulk groups on ONE queue: per-queue FIFO makes
                    # groups complete in order (the engines' packet
                    # round-robin across queues would finish concurrent
                    # groups simultaneously, stalling the first compute)
                    nc.gpsimd.dma_start(
                        cur["kv"][:], kv_d[:, int(goff[g]):int(goff[g + 1])])
                kt = cur["kv"][:, half * ROW:(half + 1) * ROW]
                r = slot_of[idx]
                qt = q_all[:, r * HQ:(r + 1) * HQ]
                for h in range(HKV):
                    nc.tensor.matmul(
                        sco[:, bi * HQ + h * G:bi * HQ + (h + 1) * G],
                        kt[:, h * D:(h + 1) * D],
                        qt[:, h * G:(h + 1) * G],
                        start=True, stop=True,
                    )
            et = epool.tile([CHUNK, bs * HQ], vdt)
            nc.scalar.activation(et[:], sco[:], EXP, bias=VSHIFT, scale=1.0)
            if pending is not None:
                emit_pv(pending)
            pending = (b0, bs, et, cur["kv"], cur["gsz"])
        emit_pv(pending)
        nc.sync.dma_start(out_d[:], out_stage[:])
    nc.compile()
    return nc


def _get_program(s_counts):
    if s_counts not in _prog_cache:
        _prog_cache[s_counts] = _build_program(s_counts)
    return _prog_cache[s_counts]


def _make_schedule(context_lens):
    L = context_lens.astype(np.int64)
    order = np.argsort(-L, kind="stable")
    s_counts = []
    for r in range(SLOTS):
        grp = order[r * NCORES:(r + 1) * NCORES]
        s_counts.append(max(1, math.ceil(int(L[grp].max()) / CHUNK)))
    return order, tuple(s_counts)


def _build_in_maps(q, k_cache, v_cache, block_tables, L, order, s_counts):
    np_k, np_v = _ndt(K_DT), _ndt(V_DT)
    C_total = sum(s_counts)
    gsizes = _group_sizes(C_total)
    nblocks_total = k_cache.shape[0]
    kf = k_cache.reshape(nblocks_total, BS, ROW)
    vf = v_cache.reshape(nblocks_total, BS, HKV, D)

    in_maps = []
    core_reqs = []
    for c in range(NCORES):
        karr = np.empty((C_total, D, ROW), np_k)
        varr = np.zeros((C_total, CHUNK, HKV, DV), np_v)
        qT = np.empty((D, SLOTS * HQ), np_k)
        reqs = []
        gc = 0
        for r in range(SLOTS):
            b = int(order[r * NCORES + c])
            reqs.append(b)
            S_r = s_counts[r]
            blocks = np.clip(block_tables[b, :S_r * BPC].astype(np.int64),
                             0, nblocks_total - 1)
            kreq = kf[blocks].reshape(S_r, CHUNK, HKV, D)
            karr[gc:gc + S_r] = \
                kreq.transpose(0, 3, 2, 1).reshape(S_r, D, ROW)
            Lb = int(L[b])
            nval = min(S_r * CHUNK, Lb)  # valid tokens in this slot
            vreq = vf[blocks].reshape(S_r * CHUNK, HKV, D)
            va = varr[gc:gc + S_r].reshape(S_r * CHUNK, HKV, DV)
            va[:nval, :, :D] = vreq[:nval]
            va[:nval, :, D] = 1.0
            qT[:, r * HQ:(r + 1) * HQ] = (q[b] * SCALE).T
            gc += S_r
        # pack combined K+V DMA groups: [D, (K chunks...)(V chunks...)] per
        # group, groups concatenated along the free dim
        varr2 = varr.reshape(C_total, CHUNK, ROWV)
        parts = []
        gc2 = 0
        for g_sz in gsizes:
            kg = karr[gc2:gc2 + g_sz].transpose(1, 0, 2).reshape(D, g_sz * ROW)
            vg = varr2[gc2:gc2 + g_sz].transpose(1, 0, 2) \
                .reshape(D, g_sz * ROWV)
            parts.append(kg)
            parts.append(vg)
            gc2 += g_sz
        kvh = np.ascontiguousarray(np.concatenate(parts, axis=1))
        in_maps.append({"kv": kvh, "qT": qT})
        core_reqs.append(reqs)
    return in_maps, core_reqs


def kernel(q, k_cache, v_cache, block_tables, context_lens):
    global last_results
    q = np.asarray(q, dtype=np.float32)
    k_cache = np.asarray(k_cache, dtype=np.float32)
    v_cache = np.asarray(v_cache, dtype=np.float32)
    block_tables = np.asarray(block_tables, dtype=np.int32)
    context_lens = np.asarray(context_lens, dtype=np.int32)

    L = context_lens.astype(np.int64)
    order, s_counts = _make_schedule(context_lens)
    nc = _get_program(s_counts)
    in_maps, core_reqs = _build_in_maps(
        q, k_cache, v_cache, block_tables, L, order, s_counts)

    res = run_bass_kernel_spmd(
        nc, in_maps, list(range(NCORES)),
        trace=bool(os.environ.get("KBASS_TRACE")),
    )
    last_results = res

    out = np.empty((B, HQ, D), np.float32)
    for c in range(NCORES):
        full = res.results[c]["out"].reshape(CHUNK, SLOTS, 2, DV) \
            .transpose(1, 2, 0, 3)
        for r, b in enumerate(core_reqs[c]):
            acc = np.empty((HQ, DV), np.float32)
            for h in range(HKV):
                jj = h % 4
                acc[h * G:(h + 1) * G] = \
                    full[r, 0 if h < 4 else 1, 32 * jj:32 * jj + G, :]
            den = np.maximum(acc[:, D:], 1e-30)
            out[b] = acc[:, :D] / den
    return out


# revision 12
# speedup vs baseline: 1.2537x; 1.0626x over previous
"""Paged GQA decode attention (B=64, HQ=32, HKV=8, D=128) on 8 TRN2 NeuronCores.

Strategy: data-parallel over request PIECES with host-side bin packing.
 - Each core runs the same static program: a sequence of slots with sizes
   `pattern` (in 128-token chunks). A slot accumulates attention for ONE
   contiguous piece of one request; long requests split into several pieces
   (across slots and cores) and the host sums the partial acc/den afterwards,
   which softmax's linearity over a shared exp-shift makes exact.
 - Bin packing (largest-bins-first, split-largest / exact-fit greedy over a
   locally-searched slot-size pattern) brings the per-core chunk count to the
   ceil(total/8) optimum, minimizing DMA bytes.
 - Host gathers each piece's KV blocks (honoring block_tables) into per-core
   shards: K pre-transposed to [d, l] tiles, V natural [l, d]; both bf16. The
   token-validity mask is folded into V host-side (invalid rows zeroed) and
   each kv head's V carries a 129th column holding the validity indicator, so
   the PV matmul also accumulates the softmax denominator and exp needs no
   bias/mask at all.
 - K and V for a group of chunks are packed into ONE combined [128, ...] DMA
   (up to ~4MB); all bulk groups go on the single gpsimd SWDGE queue so they
   complete strictly in order at the ~435 GB/s fabric ceiling (spreading
   across queues makes concurrent groups finish simultaneously instead).
   Small leading groups let compute start early.
 - Per chunk on device: scores[l,hq] = K_h^T.T @ qT (8 matmuls) into the
   group's PSUM batch tile; one exp per group on ScalarE; PV accumulation
   acc[hq,d+1] += E_h.T @ V_h (8 col-tiled matmuls into two PSUM banks), with
   the PV of group g emitted after QK of group g+1 so the PE never stalls on
   ScalarE. Slot drains go through VectorE into one SBUF staging tile that is
   written out by a single DMA at the end (interleaved output DMAs would
   share completion-semaphore lanes with the bulk loads and stall them).
"""

import math
import os
import random
import sys
from contextlib import ExitStack

import numpy as np
import ml_dtypes  # noqa: F401  (numpy bf16/fp8 dtypes)

for _p in ("/opt/trn_rl_repo", "/root/.axon_site/_ro/trn_rl_repo"):
    if os.path.isdir(_p) and _p not in sys.path:
        sys.path.insert(0, _p)
        break

import concourse.bass as bass  # noqa: F401
import concourse.tile as tile
from concourse import bacc, mybir
from concourse.bass_utils import run_bass_kernel_spmd

B, HQ, HKV, D, BS, MB = 64, 32, 8, 128, 16, 128
G = HQ // HKV              # 4 query heads per kv head
SCALE = 0.08838834764831845
NCORES = 8
CHUNK = 128                # tokens per chunk (= SBUF partitions)
BPC = CHUNK // BS          # blocks per chunk = 8
ROW = HKV * D              # 1024 K elements per token row
DV = D + 1                 # V head row + denominator indicator column
ROWV = HKV * DV            # 1032 V elements per token row
GRP = 8                    # max chunks per combined K+V DMA group (~4MB)
KV_BUFS = 5                # group tiles in flight
K_DT = "bf16"              # K/q dtype
V_DT = "bf16"              # V/E dtype

last_results = None        # stashed BassKernelResults for test.py

_prog_cache = {}
_sched_cache = {}


def _mdt(name):
    return {"f32": mybir.dt.float32, "bf16": mybir.dt.bfloat16,
            "fp8": mybir.dt.float8e4}[name]


def _ndt(name):
    return mybir.dt.np(_mdt(name))


def _group_sizes(C_total):
    """Small leading groups (fast compute start), 8-chunk steady state."""
    lead = []
    for s in (2, 2, 4):
        if sum(lead) + s <= C_total - GRP:
            lead.append(s)
    rest = C_total - sum(lead)
    sizes = lead + [GRP] * (rest // GRP)
    if rest % GRP:
        sizes.append(rest % GRP)
    return sizes


def _pack(pattern, sizes):
    """Assign request pieces to the 8*len(pattern) bins (desc order).

    Returns assignment dict (core, slot) -> (req, chunk_lo, n) or None if the
    pattern cannot hold all chunks."""
    order = sorted(range(len(pattern)), key=lambda r: -pattern[r])
    rem = sorted(((int(sizes[b]), b) for b in range(len(sizes))),
                 reverse=True)
    assign = {}
    for r in order:
        s = pattern[r]
        for c in range(NCORES):
            if not rem:
                assign[(c, r)] = None
                continue
            if rem[0][0] >= s:
                n, b = rem.pop(0)
                lo = int(sizes[b]) - n
                assign[(c, r)] = (b, lo, s)
                if n > s:
                    import bisect
                    bisect.insort_right(rem, (n - s, b))
                    rem.sort(reverse=True)
            else:
                hit = next((i for i, (n, _) in enumerate(rem) if n == s), 0)
                n, b = rem.pop(hit)
                lo = int(sizes[b]) - n
                assign[(c, r)] = (b, lo, n)
    if rem:
        return None
    return assign


def _find_pattern(sizes):
    """Local search for a slot-size pattern minimizing total chunks."""
    szs = sorted((int(s) for s in sizes), reverse=True)
    pat0 = tuple(szs[NCORES * r] for r in range(len(szs) // NCORES))
    best = (sum(pat0), pat0)
    rng = random.Random(1234)
    cur = list(pat0)
    lower = (sum(szs) + NCORES - 1) // NCORES
    for _ in range(4000):
        pat = cur[:]
        mv = rng.random()
        if mv < 0.4 and len(pat) > 2:
            i = rng.randrange(len(pat))
            if pat[i] > 1:
                pat[i] -= 1
            else:
                pat.pop(i)
        elif mv < 0.7:
            i = rng.randrange(len(pat))
            pat[i] += 1
        elif mv < 0.85 and len(pat) < 16:
            pat.append(rng.randint(1, 6))
        else:
            i = rng.randrange(len(pat))
            if pat[i] > 1:
                k = rng.randint(1, pat[i] - 1)
                pat[i] -= k
                pat.append(k)
        pat = tuple(sorted((p for p in pat if p > 0), reverse=True))
        if not pat or len(pat) > 16:
            continue
        if _pack(pat, sizes) is not None:
            if sum(pat) < best[0]:
                best = (sum(pat), pat)
                cur = list(pat)
                if best[0] <= lower:
                    break
            elif rng.random() < 0.3:
                cur = list(pat)
    return best[1]


def _make_schedule(context_lens):
    key = context_lens.tobytes()
    if key not in _sched_cache:
        sizes = np.maximum(1, -(-context_lens.astype(np.int64) // CHUNK))
        pattern = _find_pattern(sizes)
        assign = _pack(pattern, sizes)
        _sched_cache[key] = (pattern, assign)
    return _sched_cache[key]


def _build_program(pattern):
    f32 = mybir.dt.float32
    kdt, vdt = _mdt(K_DT), _mdt(V_DT)
    SLOTS = len(pattern)
    C_total = sum(pattern)
    gsizes = _group_sizes(C_total)
    goff = np.cumsum([0] + [s * (ROW + ROWV) for s in gsizes])
    slot_of = []
    for r, s in enumerate(pattern):
        slot_of += [r] * s
    nc = bacc.Bacc()

    kv_d = nc.declare_dram_parameter("kv", [D, int(goff[-1])], kdt,
                                     isOutput=False)
    qT_d = nc.declare_dram_parameter("qT", [D, SLOTS * HQ], kdt, isOutput=False)
    out_d = nc.declare_dram_parameter("out", [CHUNK, SLOTS * 2 * DV], f32,
                                      isOutput=True)

    EXP = mybir.ActivationFunctionType.Exp

    with tile.TileContext(nc) as tc, ExitStack() as ctx:
        kvpool = ctx.enter_context(tc.tile_pool(name="kvp", bufs=KV_BUFS))
        epool = ctx.enter_context(tc.tile_pool(name="e", bufs=3))
        const = ctx.enter_context(tc.tile_pool(name="cst", bufs=1))
        spsum = ctx.enter_context(tc.tile_pool(name="sp", bufs=2, space="PSUM"))
        apsum = ctx.enter_context(tc.tile_pool(name="ac", bufs=2, space="PSUM"))

        q_all = const.tile([D, SLOTS * HQ], kdt)
        nc.sync.dma_start(q_all[:], qT_d[:])
        out_stage = const.tile([CHUNK, SLOTS * 2 * DV], f32)
        # dummy matmul absorbs the q_all DMA wait so the first real matmul
        # only waits on its k/v DMA.
        dmy = spsum.tile([1, 1], f32, tag="sco")
        nc.tensor.matmul(dmy[:], q_all[0:1, 0:1], q_all[0:1, 0:1],
                         start=True, stop=True)

        accs = {}

        def get_acc(r):
            if r not in accs:
                accs[r] = (apsum.tile([CHUNK, DV], f32, tag="acca",
                                      name="acca"),
                           apsum.tile([CHUNK, DV], f32, tag="accb",
                                      name="accb"))
            return accs[r]

        def drain_slot(r):
            acc_a, acc_b = accs.pop(r)
            o = r * 2 * DV
            nc.vector.tensor_copy(out_stage[:, o:o + DV], acc_a[:])
            nc.vector.tensor_copy(out_stage[:, o + DV:o + 2 * DV], acc_b[:])

        def emit_pv(pend):
            idx0, g_sz, et, kv_t = pend
            voff = g_sz * ROW
            for bi in range(g_sz):
                idx = idx0 + bi
                vt = kv_t[:, voff + bi * ROWV:voff + (bi + 1) * ROWV]
                r = slot_of[idx]
                st = idx == 0 or slot_of[idx - 1] != r
                sp = idx == C_total - 1 or slot_of[idx + 1] != r
                acc_a, acc_b = get_acc(r)
                for h in range(HKV):
                    accp = acc_a if h < 4 else acc_b
                    jj = h % 4
                    nc.tensor.matmul(
                        accp[32 * jj:32 * jj + G, :],
                        et[:, bi * HQ + h * G:bi * HQ + (h + 1) * G],
                        vt[:, h * DV:(h + 1) * DV],
                        start=st, stop=sp,
                        tile_position=(0, 32 * jj),
                    )
                if sp:
                    drain_slot(r)

        # software-pipelined: QK+exp of group g+1 are emitted before PV of
        # group g, so the PE never stalls waiting for ScalarE's exp.
        pending = None
        idx0 = 0
        for g, g_sz in enumerate(gsizes):
            kv_t = kvpool.tile([D, g_sz * (ROW + ROWV)], kdt,
                               tag="kvg", name="kvg")
            # all bulk groups on ONE queue: per-queue FIFO makes groups
            # complete in order (spread across queues, the engines' packet
            # round-robin finishes concurrent groups simultaneously)
            nc.gpsimd.dma_start(
                kv_t[:], kv_d[:, int(goff[g]):int(goff[g + 1])])
            sco = spsum.tile([CHUNK, g_sz * HQ], f32, tag="sco")
            for bi in range(g_sz):
                r = slot_of[idx0 + bi]
                kt = kv_t[:, bi * ROW:(bi + 1) * ROW]
                qt = q_all[:, r * HQ:(r + 1) * HQ]
                for h in range(HKV):
                    nc.tensor.matmul(
                        sco[:, bi * HQ + h * G:bi * HQ + (h + 1) * G],
                        kt[:, h * D:(h + 1) * D],
                        qt[:, h * G:(h + 1) * G],
                        start=True, stop=True,
                    )
            et = epool.tile([CHUNK, g_sz * HQ], vdt)
            nc.scalar.activation(et[:], sco[:], EXP, bias=0.0, scale=1.0)
            if pending is not None:
                emit_pv(pending)
            pending = (idx0, g_sz, et, kv_t)
            idx0 += g_sz
        emit_pv(pending)
        nc.sync.dma_start(out_d[:], out_stage[:])
    nc.compile()
    return nc


def _get_program(pattern):
    if pattern not in _prog_cache:
        _prog_cache[pattern] = _build_program(pattern)
    return _prog_cache[pattern]


def _build_in_maps(q, k_cache, v_cache, block_tables, L, pattern, assign):
    np_k, np_v = _ndt(K_DT), _ndt(V_DT)
    SLOTS = len(pattern)
    C_total = sum(pattern)
    gsizes = _group_sizes(C_total)
    nblocks_total = k_cache.shape[0]
    kf = k_cache.reshape(nblocks_total, BS, ROW)
    vf = v_cache.reshape(nblocks_total, BS, HKV, D)

    in_maps = []
    for c in range(NCORES):
        karr = np.empty((C_total, D, ROW), np_k)
        varr = np.zeros((C_total, CHUNK, HKV, DV), np_v)
        qT = np.zeros((D, SLOTS * HQ), np_k)
        gc = 0
        for r, s in enumerate(pattern):
            piece = assign[(c, r)]
            if piece is None:
                # fully padded slot: K from block 0, V stays zero
                blocks = np.zeros(s * BPC, np.int64)
                kreq = kf[blocks].reshape(s, CHUNK, HKV, D)
                karr[gc:gc + s] = \
                    kreq.transpose(0, 3, 2, 1).reshape(s, D, ROW)
                gc += s
                continue
            b, lo, n = piece
            bidx = np.clip(np.arange(lo * BPC, (lo + s) * BPC), 0, MB - 1)
            blocks = np.clip(block_tables[b, bidx].astype(np.int64),
                             0, nblocks_total - 1)
            kreq = kf[blocks].reshape(s, CHUNK, HKV, D)
            karr[gc:gc + s] = kreq.transpose(0, 3, 2, 1).reshape(s, D, ROW)
            # valid tokens of this piece: global idx in [lo*CHUNK, L_b)
            nval = min(n * CHUNK, max(0, int(L[b]) - lo * CHUNK))
            if nval > 0:
                vreq = vf[blocks[:n * BPC]].reshape(n * CHUNK, HKV, D)
                va = varr[gc:gc + s].reshape(s * CHUNK, HKV, DV)
                va[:nval, :, :D] = vreq[:nval]
                va[:nval, :, D] = 1.0
            qT[:, r * HQ:(r + 1) * HQ] = (q[b] * SCALE).T
            gc += s
        varr2 = varr.reshape(C_total, CHUNK, ROWV)
        parts = []
        gc2 = 0
        for g_sz in gsizes:
            kg = karr[gc2:gc2 + g_sz].transpose(1, 0, 2).reshape(D, g_sz * ROW)
            vg = varr2[gc2:gc2 + g_sz].transpose(1, 0, 2) \
                .reshape(D, g_sz * ROWV)
            parts.append(kg)
            parts.append(vg)
            gc2 += g_sz
        kvh = np.ascontiguousarray(np.concatenate(parts, axis=1))
        in_maps.append({"kv": kvh, "qT": qT})
    return in_maps


def kernel(q, k_cache, v_cache, block_tables, context_lens):
    global last_results
    q = np.asarray(q, dtype=np.float32)
    k_cache = np.asarray(k_cache, dtype=np.float32)
    v_cache = np.asarray(v_cache, dtype=np.float32)
    block_tables = np.asarray(block_tables, dtype=np.int32)
    context_lens = np.asarray(context_lens, dtype=np.int32)

    L = context_lens.astype(np.int64)
    pattern, assign = _make_schedule(context_lens)
    SLOTS = len(pattern)
    nc = _get_program(pattern)
    in_maps = _build_in_maps(
        q, k_cache, v_cache, block_tables, L, pattern, assign)

    res = run_bass_kernel_spmd(
        nc, in_maps, list(range(NCORES)),
        trace=bool(os.environ.get("KBASS_TRACE")),
    )
    last_results = res

    num = np.zeros((B, HQ, D), np.float64)
    den = np.zeros((B, HQ, 1), np.float64)
    for c in range(NCORES):
        full = res.results[c]["out"].reshape(CHUNK, SLOTS, 2, DV) \
            .transpose(1, 2, 0, 3)
        for r in range(SLOTS):
            piece = assign[(c, r)]
            if piece is None:
                continue
            b = piece[0]
            for h in range(HKV):
                jj = h % 4
                strip = full[r, 0 if h < 4 else 1, 32 * jj:32 * jj + G, :]
                num[b, h * G:(h + 1) * G] += strip[:, :D]
                den[b, h * G:(h + 1) * G, 0] += strip[:, D]
    out = (num / np.maximum(den, 1e-30)).astype(np.float32)
    return out


# revision 14
# speedup vs baseline: 1.2723x; 1.0148x over previous
"""Paged GQA decode attention (B=64, HQ=32, HKV=8, D=128) on 8 TRN2 NeuronCores.

Strategy: data-parallel over request PIECES with host-side bin packing.
 - Each core runs the same static program: a sequence of slots with sizes
   `pattern` (in 128-token chunks). A slot accumulates attention for ONE
   contiguous piece of one request; long requests split into several pieces
   (across slots and cores) and the host sums the partial acc/den afterwards,
   which softmax's linearity over a shared exp-shift makes exact.
 - Bin packing (largest-bins-first, split-largest / exact-fit greedy over a
   locally-searched slot-size pattern) brings the per-core chunk count to the
   ceil(total/8) optimum, minimizing DMA bytes.
 - Host gathers each piece's KV blocks (honoring block_tables) into per-core
   shards: K pre-transposed to [d, l] tiles, V natural [l, d]; both bf16. The
   token-validity mask is folded into V host-side (invalid rows zeroed) and
   each kv head's V carries a 129th column holding the validity indicator, so
   the PV matmul also accumulates the softmax denominator and exp needs no
   bias/mask at all.
 - K and V for a group of chunks are packed into ONE combined [128, ...] DMA
   (up to ~4MB); all bulk groups go on the single gpsimd SWDGE queue so they
   complete strictly in order at the ~435 GB/s fabric ceiling (spreading
   across queues makes concurrent groups finish simultaneously instead).
   Small leading groups let compute start early.
 - Per chunk on device: scores[l,hq] = K_h^T.T @ qT (8 matmuls) into the
   group's PSUM batch tile; one exp per group on ScalarE; PV accumulation
   acc[hq,d+1] += E_h.T @ V_h (8 col-tiled matmuls into two PSUM banks), with
   the PV of group g emitted after QK of group g+1 so the PE never stalls on
   ScalarE. Slot drains go through VectorE into one SBUF staging tile that is
   written out by a single DMA at the end (interleaved output DMAs would
   share completion-semaphore lanes with the bulk loads and stall them).
"""

import math
import os
import random
import sys
from contextlib import ExitStack

import numpy as np
import ml_dtypes  # noqa: F401  (numpy bf16/fp8 dtypes)

for _p in ("/opt/trn_rl_repo", "/root/.axon_site/_ro/trn_rl_repo"):
    if os.path.isdir(_p) and _p not in sys.path:
        sys.path.insert(0, _p)
        break

import concourse.bass as bass  # noqa: F401
import concourse.tile as tile
from concourse import bacc, mybir
from concourse.bass_utils import run_bass_kernel_spmd

B, HQ, HKV, D, BS, MB = 64, 32, 8, 128, 16, 128
G = HQ // HKV              # 4 query heads per kv head
SCALE = 0.08838834764831845
NCORES = 8
CHUNK = 128                # tokens per chunk (= SBUF partitions)
BPC = CHUNK // BS          # blocks per chunk = 8
ROW = HKV * D              # 1024 K elements per token row
DV = D + 1                 # V head row + denominator indicator column
ROWV = HKV * DV            # 1032 V elements per token row
GRP = 8                    # max chunks per combined K+V DMA group (~4MB)
KV_BUFS = 5                # group tiles in flight
K_DT = "bf16"              # K/q dtype
V_DT = "bf16"              # V/E dtype

last_results = None        # stashed BassKernelResults for test.py

_prog_cache = {}
_sched_cache = {}


def _mdt(name):
    return {"f32": mybir.dt.float32, "bf16": mybir.dt.bfloat16,
            "fp8": mybir.dt.float8e4}[name]


def _ndt(name):
    return mybir.dt.np(_mdt(name))


def _group_sizes(C_total):
    """Small leading groups (fast compute start), 8-chunk steady state,
    small trailing groups (the last group's descriptors drain with shallow
    rings and exposed per-descriptor latency — keep it tiny)."""
    lead, tail = [], []
    for s in (2, 2, 4):
        if sum(lead) + s <= C_total - GRP:
            lead.append(s)
    for s in (2, 1):
        if sum(lead) + sum(tail) + s <= C_total - GRP:
            tail.insert(0, s)
    rest = C_total - sum(lead) - sum(tail)
    sizes = lead + [GRP] * (rest // GRP)
    if rest % GRP:
        sizes.append(rest % GRP)
    return sizes + tail


def _pack(pattern, sizes):
    """Assign request pieces to the 8*len(pattern) bins (desc order).

    Returns assignment dict (core, slot) -> (req, chunk_lo, n) or None if the
    pattern cannot hold all chunks."""
    order = sorted(range(len(pattern)), key=lambda r: -pattern[r])
    rem = sorted(((int(sizes[b]), b) for b in range(len(sizes))),
                 reverse=True)
    assign = {}
    for r in order:
        s = pattern[r]
        for c in range(NCORES):
            if not rem:
                assign[(c, r)] = None
                continue
            if rem[0][0] >= s:
                n, b = rem.pop(0)
                lo = int(sizes[b]) - n
                assign[(c, r)] = (b, lo, s)
                if n > s:
                    import bisect
                    bisect.insort_right(rem, (n - s, b))
                    rem.sort(reverse=True)
            else:
                hit = next((i for i, (n, _) in enumerate(rem) if n == s), 0)
                n, b = rem.pop(hit)
                lo = int(sizes[b]) - n
                assign[(c, r)] = (b, lo, n)
    if rem:
        return None
    return assign


def _find_pattern(sizes):
    """Local search for a slot-size pattern minimizing total chunks."""
    szs = sorted((int(s) for s in sizes), reverse=True)
    pat0 = tuple(szs[NCORES * r] for r in range(len(szs) // NCORES))
    best = (sum(pat0), pat0)
    rng = random.Random(1234)
    cur = list(pat0)
    lower = (sum(szs) + NCORES - 1) // NCORES
    for _ in range(4000):
        pat = cur[:]
        mv = rng.random()
        if mv < 0.4 and len(pat) > 2:
            i = rng.randrange(len(pat))
            if pat[i] > 1:
                pat[i] -= 1
            else:
                pat.pop(i)
        elif mv < 0.7:
            i = rng.randrange(len(pat))
            pat[i] += 1
        elif mv < 0.85 and len(pat) < 16:
            pat.append(rng.randint(1, 6))
        else:
            i = rng.randrange(len(pat))
            if pat[i] > 1:
                k = rng.randint(1, pat[i] - 1)
                pat[i] -= k
                pat.append(k)
        pat = tuple(sorted((p for p in pat if p > 0), reverse=True))
        if not pat or len(pat) > 16:
            continue
        if _pack(pat, sizes) is not None:
            if sum(pat) < best[0]:
                best = (sum(pat), pat)
                cur = list(pat)
                if best[0] <= lower:
                    break
            elif rng.random() < 0.3:
                cur = list(pat)
    return best[1]


def _make_schedule(context_lens):
    key = context_lens.tobytes()
    if key not in _sched_cache:
        sizes = np.maximum(1, -(-context_lens.astype(np.int64) // CHUNK))
        pattern = _find_pattern(sizes)
        assign = _pack(pattern, sizes)
        _sched_cache[key] = (pattern, assign)
    return _sched_cache[key]


def _build_program(pattern):
    f32 = mybir.dt.float32
    kdt, vdt = _mdt(K_DT), _mdt(V_DT)
    SLOTS = len(pattern)
    C_total = sum(pattern)
    gsizes = _group_sizes(C_total)
    goff = np.cumsum([0] + [s * (ROW + ROWV) for s in gsizes])
    slot_of = []
    for r, s in enumerate(pattern):
        slot_of += [r] * s
    nc = bacc.Bacc()

    kv_d = nc.declare_dram_parameter("kv", [D, int(goff[-1])], kdt,
                                     isOutput=False)
    qT_d = nc.declare_dram_parameter("qT", [D, SLOTS * HQ], kdt, isOutput=False)
    out_d = nc.declare_dram_parameter("out", [CHUNK, SLOTS * 2 * DV], f32,
                                      isOutput=True)

    EXP = mybir.ActivationFunctionType.Exp

    with tile.TileContext(nc) as tc, ExitStack() as ctx:
        kvpool = ctx.enter_context(tc.tile_pool(name="kvp", bufs=KV_BUFS))
        epool = ctx.enter_context(tc.tile_pool(name="e", bufs=3))
        const = ctx.enter_context(tc.tile_pool(name="cst", bufs=1))
        spsum = ctx.enter_context(tc.tile_pool(name="sp", bufs=2, space="PSUM"))
        apsum = ctx.enter_context(tc.tile_pool(name="ac", bufs=2, space="PSUM"))

        q_all = const.tile([D, SLOTS * HQ], kdt)
        nc.sync.dma_start(q_all[:], qT_d[:])
        out_stage = const.tile([CHUNK, SLOTS * 2 * DV], f32)
        # dummy matmul absorbs the q_all DMA wait so the first real matmul
        # only waits on its k/v DMA.
        dmy = spsum.tile([1, 1], f32, tag="sco")
        nc.tensor.matmul(dmy[:], q_all[0:1, 0:1], q_all[0:1, 0:1],
                         start=True, stop=True)

        accs = {}
        # slots drain in order; flush the first H slots' outputs mid-stream
        # so the final output DMA is small
        cum = 0
        H = 0
        for r, sz in enumerate(pattern):
            cum += sz
            if cum >= (6 * C_total) // 10:
                H = r + 1
                break

        def get_acc(r):
            if r not in accs:
                accs[r] = (apsum.tile([CHUNK, DV], f32, tag="acca",
                                      name="acca"),
                           apsum.tile([CHUNK, DV], f32, tag="accb",
                                      name="accb"))
            return accs[r]

        def drain_slot(r):
            acc_a, acc_b = accs.pop(r)
            o = r * 2 * DV
            nc.vector.tensor_copy(out_stage[:, o:o + DV], acc_a[:])
            nc.vector.tensor_copy(out_stage[:, o + DV:o + 2 * DV], acc_b[:])
            if r == H - 1:
                nc.sync.dma_start(out_d[:, :H * 2 * DV],
                                  out_stage[:, :H * 2 * DV])

        def emit_pv(pend):
            idx0, g_sz, et, kv_t = pend
            voff = g_sz * ROW
            for bi in range(g_sz):
                idx = idx0 + bi
                vt = kv_t[:, voff + bi * ROWV:voff + (bi + 1) * ROWV]
                r = slot_of[idx]
                st = idx == 0 or slot_of[idx - 1] != r
                sp = idx == C_total - 1 or slot_of[idx + 1] != r
                acc_a, acc_b = get_acc(r)
                for h in range(HKV):
                    accp = acc_a if h < 4 else acc_b
                    jj = h % 4
                    nc.tensor.matmul(
                        accp[32 * jj:32 * jj + G, :],
                        et[:, bi * HQ + h * G:bi * HQ + (h + 1) * G],
                        vt[:, h * DV:(h + 1) * DV],
                        start=st, stop=sp,
                        tile_position=(0, 32 * jj),
                    )
                if sp:
                    drain_slot(r)

        # software-pipelined: QK+exp of group g+1 are emitted before PV of
        # group g, so the PE never stalls waiting for ScalarE's exp.
        pending = None
        idx0 = 0
        for g, g_sz in enumerate(gsizes):
            kv_t = kvpool.tile([D, g_sz * (ROW + ROWV)], kdt,
                               tag="kvg", name="kvg")
            # all bulk groups on ONE queue: per-queue FIFO makes groups
            # complete in order (spread across queues, the engines' packet
            # round-robin finishes concurrent groups simultaneously)
            nc.gpsimd.dma_start(
                kv_t[:], kv_d[:, int(goff[g]):int(goff[g + 1])])
            sco = spsum.tile([CHUNK, g_sz * HQ], f32, tag="sco")
            for bi in range(g_sz):
                r = slot_of[idx0 + bi]
                kt = kv_t[:, bi * ROW:(bi + 1) * ROW]
                qt = q_all[:, r * HQ:(r + 1) * HQ]
                for h in range(HKV):
                    nc.tensor.matmul(
                        sco[:, bi * HQ + h * G:bi * HQ + (h + 1) * G],
                        kt[:, h * D:(h + 1) * D],
                        qt[:, h * G:(h + 1) * G],
                        start=True, stop=True,
                    )
            et = epool.tile([CHUNK, g_sz * HQ], vdt)
            nc.scalar.activation(et[:], sco[:], EXP, bias=0.0, scale=1.0)
            if pending is not None:
                emit_pv(pending)
            pending = (idx0, g_sz, et, kv_t)
            idx0 += g_sz
        emit_pv(pending)
        nc.sync.dma_start(out_d[:, H * 2 * DV:], out_stage[:, H * 2 * DV:])
    nc.compile()
    return nc


def _get_program(pattern):
    if pattern not in _prog_cache:
        _prog_cache[pattern] = _build_program(pattern)
    return _prog_cache[pattern]


def _build_in_maps(q, k_cache, v_cache, block_tables, L, pattern, assign):
    np_k, np_v = _ndt(K_DT), _ndt(V_DT)
    SLOTS = len(pattern)
    C_total = sum(pattern)
    gsizes = _group_sizes(C_total)
    nblocks_total = k_cache.shape[0]
    kf = k_cache.reshape(nblocks_total, BS, ROW)
    vf = v_cache.reshape(nblocks_total, BS, HKV, D)

    in_maps = []
    for c in range(NCORES):
        karr = np.empty((C_total, D, ROW), np_k)
        varr = np.zeros((C_total, CHUNK, HKV, DV), np_v)
        qT = np.zeros((D, SLOTS * HQ), np_k)
        gc = 0
        for r, s in enumerate(pattern):
            piece = assign[(c, r)]
            if piece is None:
                # fully padded slot: K from block 0, V stays zero
                blocks = np.zeros(s * BPC, np.int64)
                kreq = kf[blocks].reshape(s, CHUNK, HKV, D)
                karr[gc:gc + s] = \
                    kreq.transpose(0, 3, 2, 1).reshape(s, D, ROW)
                gc += s
                continue
            b, lo, n = piece
            bidx = np.clip(np.arange(lo * BPC, (lo + s) * BPC), 0, MB - 1)
            blocks = np.clip(block_tables[b, bidx].astype(np.int64),
                             0, nblocks_total - 1)
            kreq = kf[blocks].reshape(s, CHUNK, HKV, D)
            karr[gc:gc + s] = kreq.transpose(0, 3, 2, 1).reshape(s, D, ROW)
            # valid tokens of this piece: global idx in [lo*CHUNK, L_b)
            nval = min(n * CHUNK, max(0, int(L[b]) - lo * CHUNK))
            if nval > 0:
                vreq = vf[blocks[:n * BPC]].reshape(n * CHUNK, HKV, D)
                va = varr[gc:gc + s].reshape(s * CHUNK, HKV, DV)
                va[:nval, :, :D] = vreq[:nval]
                va[:nval, :, D] = 1.0
            qT[:, r * HQ:(r + 1) * HQ] = (q[b] * SCALE).T
            gc += s
        varr2 = varr.reshape(C_total, CHUNK, ROWV)
        parts = []
        gc2 = 0
        for g_sz in gsizes:
            kg = karr[gc2:gc2 + g_sz].transpose(1, 0, 2).reshape(D, g_sz * ROW)
            vg = varr2[gc2:gc2 + g_sz].transpose(1, 0, 2) \
                .reshape(D, g_sz * ROWV)
            parts.append(kg)
            parts.append(vg)
            gc2 += g_sz
        kvh = np.ascontiguousarray(np.concatenate(parts, axis=1))
        in_maps.append({"kv": kvh, "qT": qT})
    return in_maps


def kernel(q, k_cache, v_cache, block_tables, context_lens):
    global last_results
    q = np.asarray(q, dtype=np.float32)
    k_cache = np.asarray(k_cache, dtype=np.float32)
    v_cache = np.asarray(v_cache, dtype=np.float32)
    block_tables = np.asarray(block_tables, dtype=np.int32)
    context_lens = np.asarray(context_lens, dtype=np.int32)

    L = context_lens.astype(np.int64)
    pattern, assign = _make_schedule(context_lens)
    SLOTS = len(pattern)
    nc = _get_program(pattern)
    in_maps = _build_in_maps(
        q, k_cache, v_cache, block_tables, L, pattern, assign)

    res = run_bass_kernel_spmd(
        nc, in_maps, list(range(NCORES)),
        trace=bool(os.environ.get("KBASS_TRACE")),
    )
    last_results = res

    num = np.zeros((B, HQ, D), np.float64)
    den = np.zeros((B, HQ, 1), np.float64)
    for c in range(NCORES):
        full = res.results[c]["out"].reshape(CHUNK, SLOTS, 2, DV) \
            .transpose(1, 2, 0, 3)
        for r in range(SLOTS):
            piece = assign[(c, r)]
            if piece is None:
                continue
            b = piece[0]
            for h in range(HKV):
                jj = h % 4
                strip = full[r, 0 if h < 4 else 1, 32 * jj:32 * jj + G, :]
                num[b, h * G:(h + 1) * G] += strip[:, :D]
                den[b, h * G:(h + 1) * G, 0] += strip[:, D]
    out = (num / np.maximum(den, 1e-30)).astype(np.float32)
    return out
